# revision 7
# baseline (speedup 1.0000x reference)
"""AdaConv2D Trainium2 Bass kernel.

Problem (per sample): instance-norm(x) -> grouped 3x3 conv (128 groups,
2ch/group, per-sample weights) -> grouped 1x1 conv -> +bias.
B=8, Cin=Cout=256, H=W=128.

Strategy: pure data-parallel, 1 sample per NeuronCore (8 cores).

Per-core algorithm:
  - The 1x1 grouped conv is folded into the 3x3 weights:
        w_eff[co, j, t] = sum_i pw[co, i] * dw[2*(co//2)+i, j, t]
  - The instance norm is folded into weights + bias:
        w'[co, j, t] = w_eff[co, j, t] * scale[cin(co,j)]
        bias'[co]    = bias[co] - sum_{j,t} w_eff[co,j,t]*scale_j*mean_j
    where scale_c = 1/(sqrt(var_c)+eps), and the padded border cells hold
    mean_c so that (border - mean)*scale = 0 matches the reference's
    zero-padded normalized input.
  - The grouped 3x3 conv then runs on the TensorEngine as 9 shifted
    block-diagonal (2x2 blocks) 128x128 bf16 matmuls accumulated in PSUM,
    one pass per tap, channels on partitions (two halves of 128 channels).
  - The block-diagonal lhsT matrices are materialized by scattering the
    computed weights into a zero-initialized DRAM scratch (inline const)
    with a strided DMA, then loading dense [128,128] tiles back.
"""

import sys

sys.path.insert(0, "/opt/trn_rl_repo")

from contextlib import ExitStack

import numpy as np
import ml_dtypes

from concourse import bacc, bass, mybir, tile
from concourse.bass_utils import run_bass_kernel_spmd

F32 = mybir.dt.float32
BF16 = mybir.dt.bfloat16
AX = mybir.AxisListType
OP = mybir.AluOpType
ACTF = mybir.ActivationFunctionType

C = 256          # channels (per sample)
H = W = 128      # spatial
P = 128          # partitions
HP = H + 2       # padded rows/cols (130)
NHF = 2          # channel halves
CHUNK_ROWS = 16  # rows per input DMA chunk
NCHUNK = H // CHUNK_ROWS          # 8 chunks per half
ROWS_PER_MM = 4                   # output rows per psum tile (4*128=512)
SB_TILES = 4                      # psum tiles per superblock
SB_ROWS = ROWS_PER_MM * SB_TILES  # 16 rows per superblock
NSB = H // SB_ROWS                # 8 superblocks per half
NPIX = H * W
EPS = 1e-7

_CACHED = {}


def build_nc():
    nc = bacc.Bacc(trn_type="TRN2")

    x_ext = nc.declare_dram_parameter("x", [C, H, W], F32, isOutput=False)
    dw_ext = nc.declare_dram_parameter("dw_kernels", [C, 2, 3, 3], F32, isOutput=False)
    pw_ext = nc.declare_dram_parameter("pw_kernels", [C, 2, 1, 1], F32, isOutput=False)
    b_ext = nc.declare_dram_parameter("biases", [C], F32, isOutput=False)
    out_ext = nc.declare_dram_parameter("out", [C, H, W], F32, isOutput=True)

    # zero-initialized DRAM scratch for the block-diag weight matrices;
    # runtime scatter only writes the (fixed) nonzero positions, so reuse
    # across executions is idempotent.  layout: [hf, tap, ci, co] bf16
    lhsT_dram = nc.inline_tensor(
        np.zeros((NHF, 9, P, P), dtype=ml_dtypes.bfloat16), name="lhsT_zero"
    )

    with tile.TileContext(nc) as tc, ExitStack() as ctx:
        const_pool = ctx.enter_context(tc.tile_pool(name="const", bufs=1))
        chunk_pool = ctx.enter_context(tc.tile_pool(name="chunk", bufs=4))
        sq_pool = ctx.enter_context(tc.tile_pool(name="sq", bufs=2))
        psum_pool = ctx.enter_context(tc.tile_pool(name="psum", bufs=8, space="PSUM"))
        stage_pool = ctx.enter_context(tc.tile_pool(name="stage", bufs=3))
        dram_pool = ctx.enter_context(tc.tile_pool(name="drsc", bufs=1, space="DRAM"))

        # ---------------- persistent tiles ----------------
        # padded raw input (bf16), one tile per channel-half
        xnp = [
            const_pool.tile([P, HP, HP], BF16, name=f"xnp{hf}") for hf in range(NHF)
        ]
        sums = const_pool.tile([P, NHF, NCHUNK], F32, name="sums")
        sumsqs = const_pool.tile([P, NHF, NCHUNK], F32, name="sumsqs")

        # channel-layout stats (partition = channel within half)
        mean_ch = const_pool.tile([P, NHF], F32, name="mean_ch")
        scale_ch = const_pool.tile([P, NHF], F32, name="scale_ch")
        # group-layout stats (partition = group, free = j)
        mean_g = const_pool.tile([P, 2], F32, name="mean_g")
        scale_g = const_pool.tile([P, 2], F32, name="scale_g")

        # group-layout weights (partition = group)
        dwg = const_pool.tile([P, 2, 2, 9], F32, name="dwg")    # [g, i, j, t]
        pwg = const_pool.tile([P, 2, 2], F32, name="pwg")       # [g, o, i]
        biasg = const_pool.tile([P, 2], F32, name="biasg")      # [g, o]
        weff = const_pool.tile([P, 2, 2, 9], F32, name="weff")  # [g, o, j, t]
        wp = const_pool.tile([P, 2, 2, 9], F32, name="wp")      # w' f32
        wpb = const_pool.tile([P, 2, 2, 9], BF16, name="wpb")   # w' bf16
        btmp = const_pool.tile([P, 2, 2, 9], F32, name="btmp")
        bred = const_pool.tile([P, 2], F32, name="bred")
        biasp_g = const_pool.tile([P, 2], F32, name="biasp_g")  # bias' [g, o]
        biasp_ch = const_pool.tile([P, NHF], F32, name="biasp_ch")

        # small scratch for stats math
        st_a = const_pool.tile([P, NHF], F32, name="st_a")
        st_b = const_pool.tile([P, NHF], F32, name="st_b")

        # dense block-diag weights in SBUF: [ci, hf, tap, co]
        lhsT_sb = const_pool.tile([P, NHF, 9, P], BF16, name="lhsT_sb")

        # DRAM bounce buffers for cross-layout moves
        stats_dram = dram_pool.tile([2, C], F32, name="stats_dram")  # [kind, ch]
        biasp_dram = dram_pool.tile([C], F32, name="biasp_dram")

        # ---------------- weight/bias/stat DMAs (group layout) ----------------
        # dw [256,2,3,3] -> dwg[g, i, j, t]: flat ch stride 18
        nc.sync.dma_start(
            out=dwg[:],
            in_=bass.AP(tensor=dw_ext, offset=0, ap=[[36, P], [18, 2], [9, 2], [1, 9]]),
        )
        # pw [256,2,1,1] -> pwg[g, o, i]
        nc.sync.dma_start(
            out=pwg[:],
            in_=bass.AP(tensor=pw_ext, offset=0, ap=[[4, P], [2, 2], [1, 2]]),
        )
        # bias [256] -> biasg[g, o]
        nc.sync.dma_start(
            out=biasg[:],
            in_=bass.AP(tensor=b_ext, offset=0, ap=[[2, P], [1, 2]]),
        )

        # ---------------- input stream: DMA + stats + bf16 placement ----------------
        for hf in range(NHF):
            for ck in range(NCHUNK):
                chv = chunk_pool.tile([P, CHUNK_ROWS, W], F32, name="chv")
                nc.sync.dma_start(
                    out=chv[:],
                    in_=x_ext[
                        hf * P : (hf + 1) * P,
                        ck * CHUNK_ROWS : (ck + 1) * CHUNK_ROWS,
                        :,
                    ],
                )
                # convert f32 -> bf16 into padded interior; accumulate sum
                nc.vector.tensor_scalar(
                    out=xnp[hf][
                        :, 1 + ck * CHUNK_ROWS : 1 + (ck + 1) * CHUNK_ROWS, 1 : 1 + W
                    ],
                    in0=chv[:],
                    scalar1=1.0,
                    scalar2=None,
                    op0=OP.mult,
                    op1=OP.add,
                    accum_out=sums[:, hf, ck : ck + 1],
                )
                # sum of squares via ScalarE
                sq = sq_pool.tile([P, CHUNK_ROWS, W], F32, name="sq")
                nc.scalar.activation(
                    out=sq[:],
                    in_=chv[:],
                    func=ACTF.Square,
                    accum_out=sumsqs[:, hf, ck : ck + 1],
                )

        # ---------------- stats finalize (channel layout) ----------------
        inv_n = 1.0 / NPIX
        for hf in range(NHF):
            # mean = sum/n
            nc.vector.tensor_reduce(
                out=st_a[:, hf : hf + 1], in_=sums[:, hf, :], axis=AX.X, op=OP.add
            )
            nc.vector.tensor_scalar(
                out=mean_ch[:, hf : hf + 1],
                in0=st_a[:, hf : hf + 1],
                scalar1=inv_n,
                scalar2=None,
                op0=OP.mult,
            )
            # sumsq total
            nc.vector.tensor_reduce(
                out=st_a[:, hf : hf + 1], in_=sumsqs[:, hf, :], axis=AX.X, op=OP.add
            )
            # m2 = mean*mean
            nc.vector.tensor_tensor(
                out=st_b[:, hf : hf + 1],
                in0=mean_ch[:, hf : hf + 1],
                in1=mean_ch[:, hf : hf + 1],
                op=OP.mult,
            )
            # var*(n-1) = sumsq - n*m2
            nc.vector.scalar_tensor_tensor(
                out=st_b[:, hf : hf + 1],
                in0=st_b[:, hf : hf + 1],
                scalar=float(-NPIX),
                in1=st_a[:, hf : hf + 1],
                op0=OP.mult,
                op1=OP.add,
            )
            # var (bessel) = /(n-1)
            nc.vector.tensor_scalar(
                out=st_b[:, hf : hf + 1],
                in0=st_b[:, hf : hf + 1],
                scalar1=1.0 / (NPIX - 1),
                scalar2=None,
                op0=OP.mult,
            )
            # std = sqrt(var) + eps
            nc.scalar.sqrt(st_b[:, hf : hf + 1], st_b[:, hf : hf + 1])
            nc.vector.tensor_scalar(
                out=st_b[:, hf : hf + 1],
                in0=st_b[:, hf : hf + 1],
                scalar1=EPS,
                scalar2=None,
                op0=OP.add,
            )
            # scale = 1/std
            nc.vector.reciprocal(scale_ch[:, hf : hf + 1], st_b[:, hf : hf + 1])

        # ---------------- channel -> group layout (via DRAM bounce) ----------------
        for hf in range(NHF):
            nc.sync.dma_start(
                out=bass.AP(tensor=stats_dram.tensor, offset=hf * P, ap=[[1, P]]),
                in_=scale_ch[:, hf : hf + 1],
            )
            nc.sync.dma_start(
                out=bass.AP(tensor=stats_dram.tensor, offset=C + hf * P, ap=[[1, P]]),
                in_=mean_ch[:, hf : hf + 1],
            )
        # scale_g[g, j] = scale[2g+j]
        nc.sync.dma_start(
            out=scale_g[:],
            in_=bass.AP(tensor=stats_dram.tensor, offset=0, ap=[[2, P], [1, 2]]),
        )
        nc.sync.dma_start(
            out=mean_g[:],
            in_=bass.AP(tensor=stats_dram.tensor, offset=C, ap=[[2, P], [1, 2]]),
        )

        # ---------------- weight math (group layout) ----------------
        for o in range(2):
            # weff[:, o] = dwg[:, 0] * pw[g, o, 0]
            nc.vector.tensor_scalar(
                out=weff[:, o],
                in0=dwg[:, 0],
                scalar1=pwg[:, o, 0:1],
                scalar2=None,
                op0=OP.mult,
            )
            # weff[:, o] += dwg[:, 1] * pw[g, o, 1]
            nc.vector.scalar_tensor_tensor(
                out=weff[:, o],
                in0=dwg[:, 1],
                scalar=pwg[:, o, 1:2],
                in1=weff[:, o],
                op0=OP.mult,
                op1=OP.add,
            )
        for j in range(2):
            # w' = weff * scale_j
            nc.vector.tensor_scalar(
                out=wp[:, :, j],
                in0=weff[:, :, j],
                scalar1=scale_g[:, j : j + 1],
                scalar2=None,
                op0=OP.mult,
            )
            # btmp = w' * mean_j
            nc.vector.tensor_scalar(
                out=btmp[:, :, j],
                in0=wp[:, :, j],
                scalar1=mean_g[:, j : j + 1],
                scalar2=None,
                op0=OP.mult,
            )
        nc.vector.tensor_copy(wpb[:], wp[:])
        # bias' = bias - sum_{j,t} btmp
        nc.vector.tensor_reduce(out=bred[:], in_=btmp[:], axis=AX.XY, op=OP.add)
        nc.vector.tensor_tensor(
            out=biasp_g[:], in0=biasg[:], in1=bred[:], op=OP.subtract
        )

        # bias' group -> channel layout
        nc.sync.dma_start(
            out=bass.AP(tensor=biasp_dram.tensor, offset=0, ap=[[2, P], [1, 2]]),
            in_=biasp_g[:],
        )
        for hf in range(NHF):
            nc.sync.dma_start(
                out=biasp_ch[:, hf : hf + 1],
                in_=bass.AP(tensor=biasp_dram.tensor, offset=hf * P, ap=[[1, P]]),
            )

        # ---------------- scatter w' into block-diag DRAM, load back ----------------
        # dst entry (hf, t, ci=2a+j, co=2a+o) <- wpb[64*hf + a, o, j, t]
        for hf in range(NHF):
            for t in range(9):
                for j in range(2):
                    nc.sync.dma_start(
                        out=bass.AP(
                            tensor=lhsT_dram,
                            offset=(hf * 9 + t) * P * P + j * P,
                            ap=[[2 * P + 2, 64], [1, 2]],
                        ),
                        in_=wpb[64 * hf : 64 * (hf + 1), :, j, t],
                    )
        # load back densely: lhsT_sb[ci, hf, t, co]
        nc.sync.dma_start(
            out=lhsT_sb[:],
            in_=bass.AP(
                tensor=lhsT_dram,
                offset=0,
                ap=[[P, P], [9 * P * P, NHF], [P * P, 9], [1, P]],
            ),
        )

        # ---------------- border fill with mean (bf16) ----------------
        for hf in range(NHF):
            bias_ap = mean_ch[:, hf : hf + 1]
            # stage 1: left/right interior columns
            nc.scalar.activation(
                out=xnp[hf][:, 1 : 1 + H, 0],
                in_=xnp[hf][:, 1 : 1 + H, 1],
                func=ACTF.Identity,
                bias=bias_ap,
                scale=0.0,
            )
            nc.scalar.activation(
                out=xnp[hf][:, 1 : 1 + H, HP - 1],
                in_=xnp[hf][:, 1 : 1 + H, 1],
                func=ACTF.Identity,
                bias=bias_ap,
                scale=0.0,
            )
            # stage 2: full top/bottom rows (incl. corners)
            nc.scalar.activation(
                out=xnp[hf][:, 0, :],
                in_=xnp[hf][:, 1, :],
                func=ACTF.Identity,
                bias=bias_ap,
                scale=0.0,
            )
            nc.scalar.activation(
                out=xnp[hf][:, HP - 1, :],
                in_=xnp[hf][:, 1, :],
                func=ACTF.Identity,
                bias=bias_ap,
                scale=0.0,
            )

        # ---------------- conv: 9 shifted block-diag matmuls ----------------
        for hf in range(NHF):
            for sb in range(NSB):
                ps = [
                    psum_pool.tile([P, ROWS_PER_MM, W], F32, name="ps")
                    for _ in range(SB_TILES)
                ]
                for t in range(9):
                    dy, dx = t // 3, t % 3
                    for k in range(SB_TILES):
                        h0 = sb * SB_ROWS + k * ROWS_PER_MM
                        nc.tensor.matmul(
                            ps[k][:],
                            lhsT=lhsT_sb[:, hf, t, :],
                            rhs=xnp[hf][:, h0 + dy : h0 + dy + ROWS_PER_MM, dx : dx + W],
                            start=(t == 0),
                            stop=(t == 8),
                        )
                stg = stage_pool.tile([P, SB_ROWS, W], F32, name="stg")
                for k in range(SB_TILES):
                    nc.scalar.activation(
                        out=stg[:, k * ROWS_PER_MM : (k + 1) * ROWS_PER_MM, :],
                        in_=ps[k][:],
                        func=ACTF.Identity,
                        bias=biasp_ch[:, hf : hf + 1],
                        scale=1.0,
                    )
                nc.sync.dma_start(
                    out=out_ext[
                        hf * P : (hf + 1) * P,
                        sb * SB_ROWS : (sb + 1) * SB_ROWS,
                        :,
                    ],
                    in_=stg[:],
                )

    nc.compile()
    return nc


def get_nc():
    if "nc" not in _CACHED:
        _CACHED["nc"] = build_nc()
    return _CACHED["nc"]


def kernel(x, dw_kernels, pw_kernels, biases):
    x = np.asarray(x, dtype=np.float32)
    dw_kernels = np.asarray(dw_kernels, dtype=np.float32)
    pw_kernels = np.asarray(pw_kernels, dtype=np.float32)
    biases = np.asarray(biases, dtype=np.float32)
    B = x.shape[0]
    assert B == 8

    nc = get_nc()
    in_maps = [
        {
            "x": np.ascontiguousarray(x[i]),
            "dw_kernels": np.ascontiguousarray(dw_kernels[i]),
            "pw_kernels": np.ascontiguousarray(pw_kernels[i]),
            "biases": np.ascontiguousarray(biases[i]),
        }
        for i in range(B)
    ]
    res = run_bass_kernel_spmd(nc, in_maps, core_ids=list(range(B)))
    return np.stack([res.results[i]["out"] for i in range(B)], axis=0)


# revision 12
# speedup vs baseline: 1.3518x; 1.3518x over previous
"""AdaConv2D Trainium2 Bass kernel.

Problem (per sample): instance-norm(x) -> grouped 3x3 conv (128 groups,
2ch/group, per-sample weights) -> grouped 1x1 conv -> +bias.
B=8, Cin=Cout=256, H=W=128.

Strategy: pure data-parallel, 1 sample per NeuronCore (8 cores).

Per-core algorithm:
  - The 1x1 grouped conv is folded into the 3x3 weights:
        w_eff[co, j, t] = sum_i pw[co, i] * dw[2*(co//2)+i, j, t]
  - The instance norm is folded into weights + bias:
        lhsT[ci, co] = w_eff[co, j(ci), t] * scale[ci]
        bias'[co]    = bias[co] - sum_ci,t lhsT[ci, t, co] * mean[ci]
    where scale_c = 1/(sqrt(var_c)+eps); the padded border cells hold
    mean_c so that (border - mean)*scale = 0 matches the reference's
    zero-padded normalized input.
  - The grouped 3x3 conv runs on the TensorEngine as 9 shifted
    block-diagonal (2x2 blocks) 128x128 bf16 matmuls accumulated in PSUM,
    one pass per tap, channels on partitions (two halves of 128 channels).
  - Block-diag matrices: scatter the *unscaled* w_eff into a
    zero-initialized DRAM scratch (inline const) with strided DMAs (no
    stats dependency -> overlaps the x DMA-in), load dense [128,128]
    tiles back, then scale+cast per-partition (scale is indexed by ci =
    partition).  bias' comes from 9 accumulated N=1 matmuls of the scaled
    lhsT against mean[ci].
  - Per-half pipelining: half 0's conv overlaps half 1's input DMA.
"""

import sys

sys.path.insert(0, "/opt/trn_rl_repo")

from contextlib import ExitStack

import numpy as np
import ml_dtypes

from concourse import bacc, bass, mybir, tile
from concourse.bass_utils import run_bass_kernel_spmd

F32 = mybir.dt.float32
BF16 = mybir.dt.bfloat16
AX = mybir.AxisListType
OP = mybir.AluOpType
ACTF = mybir.ActivationFunctionType

C = 256          # channels (per sample)
H = W = 128      # spatial
P = 128          # partitions
HP = H + 2       # padded rows/cols (130)
NHF = 2          # channel halves
CHUNK_ROWS = 32  # rows per input DMA chunk
NCHUNK = H // CHUNK_ROWS          # 4 chunks per half
ROWS_PER_MM = 4                   # output rows per psum tile (4*128=512)
SB_TILES = 4                      # psum tiles per superblock
SB_ROWS = ROWS_PER_MM * SB_TILES  # 16 rows per superblock
NSB = H // SB_ROWS                # 8 superblocks per half
NPIX = H * W
EPS = 1e-7

_CACHED = {}


def build_nc():
    nc = bacc.Bacc(trn_type="TRN2")

    x_ext = nc.declare_dram_parameter("x", [C, H, W], F32, isOutput=False)
    dw_ext = nc.declare_dram_parameter("dw_kernels", [C, 2, 3, 3], F32, isOutput=False)
    pw_ext = nc.declare_dram_parameter("pw_kernels", [C, 2, 1, 1], F32, isOutput=False)
    b_ext = nc.declare_dram_parameter("biases", [C], F32, isOutput=False)
    out_ext = nc.declare_dram_parameter("out", [C, H, W], F32, isOutput=True)

    # zero-initialized DRAM scratch for the block-diag weight matrices;
    # runtime scatter only writes the (fixed) nonzero positions, so reuse
    # across executions is idempotent.  layout: [hf, tap, ci, co] f32
    lhsT_dram = nc.inline_tensor(
        np.zeros((NHF, 9, P, P), dtype=np.float32), name="lhsT_zero"
    )

    with tile.TileContext(nc) as tc, ExitStack() as ctx:
        const_pool = ctx.enter_context(tc.tile_pool(name="const", bufs=1))
        chunk_pool = ctx.enter_context(tc.tile_pool(name="chunk", bufs=3))
        sq_pool = ctx.enter_context(tc.tile_pool(name="sq", bufs=2))
        psum_pool = ctx.enter_context(tc.tile_pool(name="psum", bufs=8, space="PSUM"))
        stage_pool = ctx.enter_context(tc.tile_pool(name="stage", bufs=3))

        # ---------------- persistent tiles ----------------
        xnp = [
            const_pool.tile([P, HP, HP], BF16, name=f"xnp{hf}") for hf in range(NHF)
        ]
        sums = const_pool.tile([P, NHF, NCHUNK], F32, name="sums")
        sumsqs = const_pool.tile([P, NHF, NCHUNK], F32, name="sumsqs")

        mean_ch = const_pool.tile([P, NHF], F32, name="mean_ch")
        mean_bf = const_pool.tile([P, NHF], BF16, name="mean_bf")
        scale_ch = const_pool.tile([P, NHF], F32, name="scale_ch")
        bias_ch = const_pool.tile([P, NHF], F32, name="bias_ch")
        biasp_ch = const_pool.tile([P, NHF], F32, name="biasp_ch")
        st_a = const_pool.tile([P, NHF], F32, name="st_a")
        st_b = const_pool.tile([P, NHF], F32, name="st_b")

        # group-layout weights (partition = group)
        dwg = const_pool.tile([P, 2, 2, 9], F32, name="dwg")    # [g, i, j, t]
        pwg = const_pool.tile([P, 2, 2], F32, name="pwg")       # [g, o, i]
        weff = const_pool.tile([P, 2, 2, 9], F32, name="weff")  # [g, o, j, t]

        # dense block-diag weights: raw f32 (unscaled) and scaled bf16
        lhsT_raw = const_pool.tile([P, NHF, 9, P], F32, name="lhsT_raw")
        lhsT_sb = const_pool.tile([P, NHF, 9, P], BF16, name="lhsT_sb")

        # ------------- early DMAs (no stats dependency) -------------
        nc.sync.dma_start(
            out=dwg[:],
            in_=bass.AP(tensor=dw_ext, offset=0, ap=[[36, P], [18, 2], [9, 2], [1, 9]]),
        )
        nc.sync.dma_start(
            out=pwg[:],
            in_=bass.AP(tensor=pw_ext, offset=0, ap=[[4, P], [2, 2], [1, 2]]),
        )
        # bias [256] -> bias_ch[c, hf]
        nc.sync.dma_start(
            out=bias_ch[:],
            in_=bass.AP(tensor=b_ext, offset=0, ap=[[1, P], [P, NHF]]),
        )

        # ------------- w_eff (group layout) + scatter + load -------------
        for o in range(2):
            nc.vector.tensor_scalar(
                out=weff[:, o],
                in0=dwg[:, 0],
                scalar1=pwg[:, o, 0:1],
                scalar2=None,
                op0=OP.mult,
            )
            nc.vector.scalar_tensor_tensor(
                out=weff[:, o],
                in0=dwg[:, 1],
                scalar=pwg[:, o, 1:2],
                in1=weff[:, o],
                op0=OP.mult,
                op1=OP.add,
            )
        # scatter: dst (hf, t, ci=2a+j, co=2a+o) <- weff[64*hf + a, o, j, t]
        # (DMA APs max out at 3 dims incl. the trailing unit -> one DMA
        #  per (hf, t, j) with dims (a, o))
        for hf in range(NHF):
            for t in range(9):
                for j in range(2):
                    nc.sync.dma_start(
                        out=bass.AP(
                            tensor=lhsT_dram,
                            offset=(hf * 9 + t) * P * P + j * P,
                            ap=[[2 * P + 2, 64], [1, 2]],
                        ),
                        in_=weff[64 * hf : 64 * (hf + 1), :, j, t],
                    )
        # load back densely: lhsT_raw[ci, hf, t, co]
        nc.sync.dma_start(
            out=lhsT_raw[:],
            in_=bass.AP(
                tensor=lhsT_dram,
                offset=0,
                ap=[[P, P], [9 * P * P, NHF], [P * P, 9], [1, P]],
            ),
        )

        # ------------- per-half pipeline -------------
        for hf in range(NHF):
            # --- input stream: DMA + stats + bf16 placement ---
            for ck in range(NCHUNK):
                chv = chunk_pool.tile([P, CHUNK_ROWS, W], F32, name="chv")
                nc.gpsimd.dma_start(
                    out=chv[:],
                    in_=x_ext[
                        hf * P : (hf + 1) * P,
                        ck * CHUNK_ROWS : (ck + 1) * CHUNK_ROWS,
                        :,
                    ],
                )
                # convert f32 -> bf16 into padded interior; accumulate sum
                nc.vector.tensor_scalar(
                    out=xnp[hf][
                        :, 1 + ck * CHUNK_ROWS : 1 + (ck + 1) * CHUNK_ROWS, 1 : 1 + W
                    ],
                    in0=chv[:],
                    scalar1=1.0,
                    scalar2=None,
                    op0=OP.mult,
                    op1=OP.add,
                    accum_out=sums[:, hf, ck : ck + 1],
                )
                # sum of squares via ScalarE
                sq = sq_pool.tile([P, CHUNK_ROWS, W], F32, name="sq")
                nc.scalar.activation(
                    out=sq[:],
                    in_=chv[:],
                    func=ACTF.Square,
                    accum_out=sumsqs[:, hf, ck : ck + 1],
                )

            # --- stats finalize (channel layout) ---
            nc.vector.tensor_reduce(
                out=st_a[:, hf : hf + 1], in_=sums[:, hf, :], axis=AX.X, op=OP.add
            )
            nc.vector.tensor_scalar(
                out=mean_ch[:, hf : hf + 1],
                in0=st_a[:, hf : hf + 1],
                scalar1=1.0 / NPIX,
                scalar2=None,
                op0=OP.mult,
            )
            nc.vector.tensor_reduce(
                out=st_a[:, hf : hf + 1], in_=sumsqs[:, hf, :], axis=AX.X, op=OP.add
            )
            nc.vector.tensor_tensor(
                out=st_b[:, hf : hf + 1],
                in0=mean_ch[:, hf : hf + 1],
                in1=mean_ch[:, hf : hf + 1],
                op=OP.mult,
            )
            nc.vector.scalar_tensor_tensor(
                out=st_b[:, hf : hf + 1],
                in0=st_b[:, hf : hf + 1],
                scalar=float(-NPIX),
                in1=st_a[:, hf : hf + 1],
                op0=OP.mult,
                op1=OP.add,
            )
            nc.vector.tensor_scalar(
                out=st_b[:, hf : hf + 1],
                in0=st_b[:, hf : hf + 1],
                scalar1=1.0 / (NPIX - 1),
                scalar2=None,
                op0=OP.mult,
            )
            nc.scalar.sqrt(st_b[:, hf : hf + 1], st_b[:, hf : hf + 1])
            nc.vector.tensor_scalar(
                out=st_b[:, hf : hf + 1],
                in0=st_b[:, hf : hf + 1],
                scalar1=EPS,
                scalar2=None,
                op0=OP.add,
            )
            nc.vector.reciprocal(scale_ch[:, hf : hf + 1], st_b[:, hf : hf + 1])
            nc.vector.tensor_copy(mean_bf[:, hf : hf + 1], mean_ch[:, hf : hf + 1])

            # --- scale + cast the block-diag weights (per-partition ci) ---
            nc.vector.tensor_scalar(
                out=lhsT_sb[:, hf],
                in0=lhsT_raw[:, hf],
                scalar1=scale_ch[:, hf : hf + 1],
                scalar2=None,
                op0=OP.mult,
            )

            # --- bias' = bias - lhsT^T @ mean  (9 accumulated N=1 matmuls) ---
            bps = psum_pool.tile([P, 1], F32, name="bps", bufs=2)
            for t in range(9):
                nc.tensor.matmul(
                    bps[:],
                    lhsT=lhsT_sb[:, hf, t, :],
                    rhs=mean_bf[:, hf : hf + 1],
                    start=(t == 0),
                    stop=(t == 8),
                )
            nc.vector.tensor_tensor(
                out=biasp_ch[:, hf : hf + 1],
                in0=bias_ch[:, hf : hf + 1],
                in1=bps[:],
                op=OP.subtract,
            )

            # --- border fill with mean (bf16) ---
            bias_ap = mean_ch[:, hf : hf + 1]
            nc.scalar.activation(
                out=xnp[hf][:, 1 : 1 + H, 0],
                in_=xnp[hf][:, 1 : 1 + H, 1],
                func=ACTF.Identity,
                bias=bias_ap,
                scale=0.0,
            )
            nc.scalar.activation(
                out=xnp[hf][:, 1 : 1 + H, HP - 1],
                in_=xnp[hf][:, 1 : 1 + H, 1],
                func=ACTF.Identity,
                bias=bias_ap,
                scale=0.0,
            )
            nc.scalar.activation(
                out=xnp[hf][:, 0, :],
                in_=xnp[hf][:, 1, :],
                func=ACTF.Identity,
                bias=bias_ap,
                scale=0.0,
            )
            nc.scalar.activation(
                out=xnp[hf][:, HP - 1, :],
                in_=xnp[hf][:, 1, :],
                func=ACTF.Identity,
                bias=bias_ap,
                scale=0.0,
            )

            # --- conv: 9 shifted block-diag matmuls per psum tile ---
            for sb in range(NSB):
                ps = [
                    psum_pool.tile([P, ROWS_PER_MM, W], F32, name="ps", bufs=6)
                    for _ in range(SB_TILES)
                ]
                for t in range(9):
                    dy, dx = t // 3, t % 3
                    for k in range(SB_TILES):
                        h0 = sb * SB_ROWS + k * ROWS_PER_MM
                        nc.tensor.matmul(
                            ps[k][:],
                            lhsT=lhsT_sb[:, hf, t, :],
                            rhs=xnp[hf][
                                :, h0 + dy : h0 + dy + ROWS_PER_MM, dx : dx + W
                            ],
                            start=(t == 0),
                            stop=(t == 8),
                        )
                stg = stage_pool.tile([P, SB_ROWS, W], F32, name="stg")
                for k in range(SB_TILES):
                    nc.scalar.activation(
                        out=stg[:, k * ROWS_PER_MM : (k + 1) * ROWS_PER_MM, :],
                        in_=ps[k][:],
                        func=ACTF.Identity,
                        bias=biasp_ch[:, hf : hf + 1],
                        scale=1.0,
                    )
                nc.sync.dma_start(
                    out=out_ext[
                        hf * P : (hf + 1) * P,
                        sb * SB_ROWS : (sb + 1) * SB_ROWS,
                        :,
                    ],
                    in_=stg[:],
                )

    nc.compile()
    return nc


def get_nc():
    if "nc" not in _CACHED:
        _CACHED["nc"] = build_nc()
    return _CACHED["nc"]


def kernel(x, dw_kernels, pw_kernels, biases):
    x = np.asarray(x, dtype=np.float32)
    dw_kernels = np.asarray(dw_kernels, dtype=np.float32)
    pw_kernels = np.asarray(pw_kernels, dtype=np.float32)
    biases = np.asarray(biases, dtype=np.float32)
    B = x.shape[0]
    assert B == 8

    nc = get_nc()
    in_maps = [
        {
            "x": np.ascontiguousarray(x[i]),
            "dw_kernels": np.ascontiguousarray(dw_kernels[i]),
            "pw_kernels": np.ascontiguousarray(pw_kernels[i]),
            "biases": np.ascontiguousarray(biases[i]),
        }
        for i in range(B)
    ]
    res = run_bass_kernel_spmd(nc, in_maps, core_ids=list(range(B)))
    return np.stack([res.results[i]["out"] for i in range(B)], axis=0)


# revision 18
# speedup vs baseline: 1.4047x; 1.0391x over previous
"""AdaConv2D Trainium2 Bass kernel.

Problem (per sample): instance-norm(x) -> grouped 3x3 conv (128 groups,
2ch/group, per-sample weights) -> grouped 1x1 conv -> +bias.
B=8, Cin=Cout=256, H=W=128.

Strategy: pure data-parallel, 1 sample per NeuronCore (8 cores).

Per-core algorithm:
  - The 1x1 grouped conv is folded into the 3x3 weights:
        w_eff[co, j, t] = sum_i pw[co, i] * dw[2*(co//2)+i, j, t]
  - The instance norm is folded into weights + bias:
        lhsT[ci, co] = w_eff[co, j(ci), t] * scale[ci]
        bias'[co]    = bias[co] - sum_ci,t lhsT[ci, t, co] * mean[ci]
    where scale_c = 1/(sqrt(var_c)+eps); the padded border cells hold
    mean_c so that (border - mean)*scale = 0 matches the reference's
    zero-padded normalized input.
  - The grouped 3x3 conv runs on the TensorEngine as 9 shifted
    block-diagonal (2x2 blocks) 128x128 bf16 matmuls accumulated in PSUM,
    one pass per tap, channels on partitions (two halves of 128 channels).
  - Block-diag matrices: scatter the *unscaled* w_eff into a
    zero-initialized DRAM scratch (inline const) with strided DMAs (no
    stats dependency -> overlaps the x DMA-in), load dense [128,128]
    tiles back, then scale+cast per-partition (scale is indexed by ci =
    partition).  bias' comes from 9 accumulated N=1 matmuls of the scaled
    lhsT against mean[ci].
  - Per-half pipelining: half 0's conv overlaps half 1's input DMA.
"""

import sys

sys.path.insert(0, "/opt/trn_rl_repo")

from contextlib import ExitStack

import numpy as np
import ml_dtypes

from concourse import bacc, bass, mybir, tile
from concourse.bass_utils import run_bass_kernel_spmd

F32 = mybir.dt.float32
BF16 = mybir.dt.bfloat16
AX = mybir.AxisListType
OP = mybir.AluOpType
ACTF = mybir.ActivationFunctionType

C = 256          # channels (per sample)
H = W = 128      # spatial
P = 128          # partitions
HP = H + 2       # padded rows/cols (130)
NHF = 2          # channel halves
CHUNK_ROWS = 16  # rows per input DMA chunk
NCHUNK = H // CHUNK_ROWS          # 4 chunks per half
ROWS_PER_MM = 4                   # output rows per psum tile (4*128=512)
SB_TILES = 4                      # psum tiles per superblock
SB_ROWS = ROWS_PER_MM * SB_TILES  # 16 rows per superblock
NSB = H // SB_ROWS                # 8 superblocks per half
NPIX = H * W
EPS = 1e-7

_CACHED = {}


def build_nc():
    nc = bacc.Bacc(trn_type="TRN2")

    x_ext = nc.declare_dram_parameter("x", [C, H, W], F32, isOutput=False)
    dw_ext = nc.declare_dram_parameter("dw_kernels", [C, 2, 3, 3], F32, isOutput=False)
    pw_ext = nc.declare_dram_parameter("pw_kernels", [C, 2, 1, 1], F32, isOutput=False)
    b_ext = nc.declare_dram_parameter("biases", [C], F32, isOutput=False)
    out_ext = nc.declare_dram_parameter("out", [C, H, W], F32, isOutput=True)

    # zero-initialized DRAM scratch for the block-diag weight matrices;
    # runtime scatter only writes the (fixed) nonzero positions, so reuse
    # across executions is idempotent.  layout: [ci, hf, tap, co] f32
    # (ci-major so the load back to SBUF is one big descriptor per
    # partition instead of thousands of 512B ones)
    lhsT_dram = nc.inline_tensor(
        np.zeros((P, NHF, 9, P), dtype=np.float32), name="lhsT_zero"
    )

    with tile.TileContext(nc) as tc, ExitStack() as ctx:
        const_pool = ctx.enter_context(tc.tile_pool(name="const", bufs=1))
        chunk_pool = ctx.enter_context(tc.tile_pool(name="chunk", bufs=6))
        sq_pool = ctx.enter_context(tc.tile_pool(name="sq", bufs=2))
        psum_pool = ctx.enter_context(tc.tile_pool(name="psum", bufs=8, space="PSUM"))
        stage_pool = ctx.enter_context(tc.tile_pool(name="stage", bufs=3))

        # ---------------- persistent tiles ----------------
        xnp = [
            const_pool.tile([P, HP, HP], BF16, name=f"xnp{hf}") for hf in range(NHF)
        ]
        sums = const_pool.tile([P, NHF, NCHUNK], F32, name="sums")
        sumsqs = const_pool.tile([P, NHF, NCHUNK], F32, name="sumsqs")

        mean_ch = const_pool.tile([P, NHF], F32, name="mean_ch")
        mean_bf = const_pool.tile([P, NHF], BF16, name="mean_bf")
        scale_ch = const_pool.tile([P, NHF], F32, name="scale_ch")
        bias_ch = const_pool.tile([P, NHF], F32, name="bias_ch")
        biasp_ch = const_pool.tile([P, NHF], F32, name="biasp_ch")
        st_a = const_pool.tile([P, NHF], F32, name="st_a")
        st_b = const_pool.tile([P, NHF], F32, name="st_b")

        # group-layout weights (partition = group)
        dwg = const_pool.tile([P, 2, 2, 9], F32, name="dwg")    # [g, i, j, t]
        pwg = const_pool.tile([P, 2, 2], F32, name="pwg")       # [g, o, i]
        weff = const_pool.tile([P, 2, 2, 9], F32, name="weff")  # [g, o, j, t]

        # dense block-diag weights: raw f32 (unscaled) and scaled bf16
        lhsT_raw = const_pool.tile([P, NHF, 9, P], F32, name="lhsT_raw")
        lhsT_sb = const_pool.tile([P, NHF, 9, P], BF16, name="lhsT_sb")

        # ------------- early DMAs (no stats dependency) -------------
        nc.sync.dma_start(
            out=dwg[:],
            in_=bass.AP(tensor=dw_ext, offset=0, ap=[[36, P], [18, 2], [9, 2], [1, 9]]),
        )
        nc.sync.dma_start(
            out=pwg[:],
            in_=bass.AP(tensor=pw_ext, offset=0, ap=[[4, P], [2, 2], [1, 2]]),
        )

        # ------------- w_eff (group layout) + scatter + load -------------
        for o in range(2):
            nc.vector.tensor_scalar(
                out=weff[:, o],
                in0=dwg[:, 0],
                scalar1=pwg[:, o, 0:1],
                scalar2=None,
                op0=OP.mult,
            )
            nc.vector.scalar_tensor_tensor(
                out=weff[:, o],
                in0=dwg[:, 1],
                scalar=pwg[:, o, 1:2],
                in1=weff[:, o],
                op0=OP.mult,
                op1=OP.add,
            )
        # scatter: dst (ci=2a+j, hf, t, co=2a+o) <- weff[64*hf + a, o, j, t]
        # (DMA APs max out at 3 dims incl. the trailing unit -> one DMA
        #  per (hf, t, j) with dims (a, o)); per-half: scatter then load
        CI_STRIDE = NHF * 9 * P  # 2304
        for hf in range(NHF):
            for t in range(9):
                for j in range(2):
                    nc.sync.dma_start(
                        out=bass.AP(
                            tensor=lhsT_dram,
                            offset=j * CI_STRIDE + hf * 9 * P + t * P,
                            ap=[[2 * CI_STRIDE + 2, 64], [1, 2]],
                        ),
                        in_=weff[64 * hf : 64 * (hf + 1), :, j, t],
                    )
            # load back densely: lhsT_raw[ci, hf, t, co] (contiguous 4.6KB
            # per partition)
            nc.sync.dma_start(
                out=lhsT_raw[:, hf],
                in_=bass.AP(
                    tensor=lhsT_dram,
                    offset=hf * 9 * P,
                    ap=[[CI_STRIDE, P], [P, 9], [1, P]],
                ),
            )
        # bias [256] -> bias_ch[c, hf] (after the latency-critical loads)
        nc.sync.dma_start(
            out=bias_ch[:],
            in_=bass.AP(tensor=b_ext, offset=0, ap=[[1, P], [P, NHF]]),
        )

        # ------------- per-half pipeline -------------
        for hf in range(NHF):
            # --- input stream: DMA (split across the SWDGE + ACT-HWDGE
            # rings so both move x concurrently) + stats + bf16 placement
            chunk_tiles = []
            for ck in range(NCHUNK):
                chv = chunk_pool.tile([P, CHUNK_ROWS, W], F32, name="chv")
                chunk_tiles.append(chv)
                dma_eng = nc.gpsimd if ck % 2 == 0 else nc.scalar
                dma_eng.dma_start(
                    out=chv[:],
                    in_=x_ext[
                        hf * P : (hf + 1) * P,
                        ck * CHUNK_ROWS : (ck + 1) * CHUNK_ROWS,
                        :,
                    ],
                )
            for ck in range(NCHUNK):
                chv = chunk_tiles[ck]
                # convert f32 -> bf16 into padded interior; accumulate sum
                nc.vector.tensor_scalar(
                    out=xnp[hf][
                        :, 1 + ck * CHUNK_ROWS : 1 + (ck + 1) * CHUNK_ROWS, 1 : 1 + W
                    ],
                    in0=chv[:],
                    scalar1=1.0,
                    scalar2=None,
                    op0=OP.mult,
                    op1=OP.add,
                    accum_out=sums[:, hf, ck : ck + 1],
                )
                # sum of squares via ScalarE
                sq = sq_pool.tile([P, CHUNK_ROWS, W], F32, name="sq")
                nc.scalar.activation(
                    out=sq[:],
                    in_=chv[:],
                    func=ACTF.Square,
                    accum_out=sumsqs[:, hf, ck : ck + 1],
                )

            # --- stats finalize (channel layout) ---
            nc.vector.tensor_reduce(
                out=st_a[:, hf : hf + 1], in_=sums[:, hf, :], axis=AX.X, op=OP.add
            )
            nc.vector.tensor_scalar(
                out=mean_ch[:, hf : hf + 1],
                in0=st_a[:, hf : hf + 1],
                scalar1=1.0 / NPIX,
                scalar2=None,
                op0=OP.mult,
            )
            nc.vector.tensor_reduce(
                out=st_a[:, hf : hf + 1], in_=sumsqs[:, hf, :], axis=AX.X, op=OP.add
            )
            nc.vector.tensor_tensor(
                out=st_b[:, hf : hf + 1],
                in0=mean_ch[:, hf : hf + 1],
                in1=mean_ch[:, hf : hf + 1],
                op=OP.mult,
            )
            nc.vector.scalar_tensor_tensor(
                out=st_b[:, hf : hf + 1],
                in0=st_b[:, hf : hf + 1],
                scalar=float(-NPIX),
                in1=st_a[:, hf : hf + 1],
                op0=OP.mult,
                op1=OP.add,
            )
            nc.vector.tensor_scalar(
                out=st_b[:, hf : hf + 1],
                in0=st_b[:, hf : hf + 1],
                scalar1=1.0 / (NPIX - 1),
                scalar2=None,
                op0=OP.mult,
            )
            nc.scalar.sqrt(st_b[:, hf : hf + 1], st_b[:, hf : hf + 1])
            nc.vector.tensor_scalar(
                out=st_b[:, hf : hf + 1],
                in0=st_b[:, hf : hf + 1],
                scalar1=EPS,
                scalar2=None,
                op0=OP.add,
            )
            nc.vector.reciprocal(scale_ch[:, hf : hf + 1], st_b[:, hf : hf + 1])
            nc.vector.tensor_copy(mean_bf[:, hf : hf + 1], mean_ch[:, hf : hf + 1])

            # --- scale + cast the block-diag weights (per-partition ci) ---
            nc.vector.tensor_scalar(
                out=lhsT_sb[:, hf],
                in0=lhsT_raw[:, hf],
                scalar1=scale_ch[:, hf : hf + 1],
                scalar2=None,
                op0=OP.mult,
            )

            # --- bias' = bias - lhsT^T @ mean  (9 accumulated N=1 matmuls) ---
            bps = psum_pool.tile([P, 1], F32, name="bps", bufs=2)
            for t in range(9):
                nc.tensor.matmul(
                    bps[:],
                    lhsT=lhsT_sb[:, hf, t, :],
                    rhs=mean_bf[:, hf : hf + 1],
                    start=(t == 0),
                    stop=(t == 8),
                )
            nc.vector.tensor_tensor(
                out=biasp_ch[:, hf : hf + 1],
                in0=bias_ch[:, hf : hf + 1],
                in1=bps[:],
                op=OP.subtract,
            )

            # --- border fill with mean (bf16) ---
            bias_ap = mean_ch[:, hf : hf + 1]
            nc.scalar.activation(
                out=xnp[hf][:, 1 : 1 + H, 0],
                in_=xnp[hf][:, 1 : 1 + H, 1],
                func=ACTF.Identity,
                bias=bias_ap,
                scale=0.0,
            )
            nc.scalar.activation(
                out=xnp[hf][:, 1 : 1 + H, HP - 1],
                in_=xnp[hf][:, 1 : 1 + H, 1],
                func=ACTF.Identity,
                bias=bias_ap,
                scale=0.0,
            )
            nc.scalar.activation(
                out=xnp[hf][:, 0, :],
                in_=xnp[hf][:, 1, :],
                func=ACTF.Identity,
                bias=bias_ap,
                scale=0.0,
            )
            nc.scalar.activation(
                out=xnp[hf][:, HP - 1, :],
                in_=xnp[hf][:, 1, :],
                func=ACTF.Identity,
                bias=bias_ap,
                scale=0.0,
            )

            # --- conv: 9 shifted block-diag matmuls per psum tile ---
            for sb in range(NSB):
                ps = [
                    psum_pool.tile([P, ROWS_PER_MM, W], F32, name="ps", bufs=6)
                    for _ in range(SB_TILES)
                ]
                for t in range(9):
                    dy, dx = t // 3, t % 3
                    for k in range(SB_TILES):
                        h0 = sb * SB_ROWS + k * ROWS_PER_MM
                        nc.tensor.matmul(
                            ps[k][:],
                            lhsT=lhsT_sb[:, hf, t, :],
                            rhs=xnp[hf][
                                :, h0 + dy : h0 + dy + ROWS_PER_MM, dx : dx + W
                            ],
                            start=(t == 0),
                            stop=(t == 8),
                        )
                stg = stage_pool.tile([P, SB_ROWS, W], F32, name="stg")
                for k in range(SB_TILES):
                    nc.scalar.activation(
                        out=stg[:, k * ROWS_PER_MM : (k + 1) * ROWS_PER_MM, :],
                        in_=ps[k][:],
                        func=ACTF.Identity,
                        bias=biasp_ch[:, hf : hf + 1],
                        scale=1.0,
                    )
                nc.sync.dma_start(
                    out=out_ext[
                        hf * P : (hf + 1) * P,
                        sb * SB_ROWS : (sb + 1) * SB_ROWS,
                        :,
                    ],
                    in_=stg[:],
                )

    nc.compile()
    return nc


def get_nc():
    if "nc" not in _CACHED:
        _CACHED["nc"] = build_nc()
    return _CACHED["nc"]


def kernel(x, dw_kernels, pw_kernels, biases):
    x = np.asarray(x, dtype=np.float32)
    dw_kernels = np.asarray(dw_kernels, dtype=np.float32)
    pw_kernels = np.asarray(pw_kernels, dtype=np.float32)
    biases = np.asarray(biases, dtype=np.float32)
    B = x.shape[0]
    assert B == 8

    nc = get_nc()
    in_maps = [
        {
            "x": np.ascontiguousarray(x[i]),
            "dw_kernels": np.ascontiguousarray(dw_kernels[i]),
            "pw_kernels": np.ascontiguousarray(pw_kernels[i]),
            "biases": np.ascontiguousarray(biases[i]),
        }
        for i in range(B)
    ]
    res = run_bass_kernel_spmd(nc, in_maps, core_ids=list(range(B)))
    return np.stack([res.results[i]["out"] for i in range(B)], axis=0)
